# revision 1
# baseline (speedup 1.0000x reference)
"""Trainium2 Bass kernel for nn_CausalTransformerEncoder.

Sharding: 8 cores = 2 (batch) x 4 (sequence chunks of 512 tokens).
Per layer: each core computes LN + QKV for its own 512 tokens, AllGathers
K^T/V (bf16) within its 4-core batch group, runs attention over the full
key range (block-causal handled by per-core bias tables + a local diagonal
pass with static triangular masks), then out-proj, LN2 and FFN on its own
tokens. Residual stream stays fp32 in SBUF; matmuls run in bf16.

Softmax: exp without max subtraction (scores are small; validated on host),
denominator for free via a ones-column appended to V (matmul M=65 outputs
o rows 0:64 and the sum at row 64). Key-padding from `lengths` is handled
by zeroing V rows + ones-column at the source, so masked keys drop out of
both numerator and denominator with no extra masking work.
"""

import sys
import os

for _p in ("/opt/trn_rl_repo", os.path.expanduser("~/.axon_site/_ro/trn_rl_repo")):
    if os.path.isdir(_p) and _p not in sys.path:
        sys.path.insert(0, _p)

import numpy as np
import ml_dtypes

import concourse.bass as bass
from concourse import bacc
import concourse.mybir as mybir
import concourse.tile as tile
from concourse.bass import ts
from concourse.bass_utils import run_bass_kernel_spmd

F32 = mybir.dt.float32
BF16 = mybir.dt.bfloat16
AF = mybir.ActivationFunctionType
AX = mybir.AxisListType

B, T, D, H, L, FF = 2, 2048, 1024, 16, 4, 4096
DH = D // H          # 64
P = 128
CH = 512             # tokens per core
NT = CH // P         # 4 token tiles per core
NG = D // P          # 8 feature sub-tiles
NF = FF // P         # 32 ffn sub-tiles
NHP = H // 2         # 8 head pairs
NCHUNK = 4           # sequence chunks per batch group
SCALE = 1.0 / np.sqrt(DH)
NEG = -1.0e9

_CACHE = {}


def _build(with_bias: bool, nlayers: int = L, dump: bool = False, for_sim: bool = False):
    nc = bacc.Bacc("TRN2", target_bir_lowering=False, debug=False, num_devices=8)

    # ---------------- I/O ----------------
    x0 = nc.dram_tensor("x0", [CH, D], F32, kind="ExternalInput")
    wq = nc.dram_tensor("wq", [L, D, D], BF16, kind="ExternalInput")
    wk = nc.dram_tensor("wk", [L, D, D], BF16, kind="ExternalInput")
    wv = nc.dram_tensor("wv", [L, D, D], BF16, kind="ExternalInput")
    wo = nc.dram_tensor("wo", [L, D, D], BF16, kind="ExternalInput")
    w1 = nc.dram_tensor("w1", [L, D, FF], BF16, kind="ExternalInput")
    w2 = nc.dram_tensor("w2", [L, FF, D], BF16, kind="ExternalInput")
    bqk = nc.dram_tensor("bqk", [L, P, 16], F32, kind="ExternalInput")
    b1c = nc.dram_tensor("b1c", [L, P, NF], F32, kind="ExternalInput")
    mdense = nc.dram_tensor("mdense", [P, NCHUNK], F32, kind="ExternalInput")
    padcol = nc.dram_tensor("padcol", [P, NT], F32, kind="ExternalInput")
    padfull = nc.dram_tensor("padfull", [P, NCHUNK, NT], F32, kind="ExternalInput")
    trim = nc.dram_tensor("trim", [NT, P, CH], BF16, kind="ExternalInput")
    ident = nc.dram_tensor("ident", [P, P], BF16, kind="ExternalInput")
    if with_bias:
        brows = nc.dram_tensor("brows", [3, D], BF16, kind="ExternalInput")
    y = nc.dram_tensor("y", [CH, D], F32, kind="ExternalOutput")
    if dump:
        dbg_hT = nc.dram_tensor("dbg_hT", [P, NG, CH], BF16, kind="ExternalOutput")
        dbg_qT = nc.dram_tensor("dbg_qT", [P, NHP, CH], BF16, kind="ExternalOutput")
        dbg_kT = nc.dram_tensor("dbg_kT", [P, NG, CH], BF16, kind="ExternalOutput")
        dbg_vloc = nc.dram_tensor(
            "dbg_vloc", [P, NT, H, DH + 1], BF16, kind="ExternalOutput"
        )
        dbg_kc = nc.dram_tensor(
            "dbg_kc", [P, NCHUNK, NG, CH], BF16, kind="ExternalOutput"
        )
        dbg_vc = nc.dram_tensor(
            "dbg_vc", [P, NCHUNK, NT, H, DH + 1], BF16, kind="ExternalOutput"
        )
        dbg_oT = nc.dram_tensor("dbg_oT", [P, NHP, CH], BF16, kind="ExternalOutput")
        dbg_Rb = nc.dram_tensor("dbg_Rb", [P, NHP, CH], BF16, kind="ExternalOutput")
        dbg_x1 = nc.dram_tensor("dbg_x1", [P, NT, D], F32, kind="ExternalOutput")

    # per-layer DRAM for the collective
    kvin = [nc.dram_tensor(f"kvin{l}", [2, P, NG * CH], BF16) for l in range(L)]
    kvall = [
        nc.dram_tensor(f"kvall{l}", [NCHUNK, 2, P, NG * CH], BF16)
        for l in range(L)
    ]

    # ---------------- persistent SBUF ----------------
    xs = nc.alloc_sbuf_tensor("xs", [P, NT, D], F32).ap()
    hT = nc.alloc_sbuf_tensor("hT", [P, NG, CH], BF16).ap()
    oT = nc.alloc_sbuf_tensor("oT", [P, NHP, CH], BF16).ap()
    oTb = nc.alloc_sbuf_tensor("oTb", [64, NHP, CH], BF16).ap()
    RbA = nc.alloc_sbuf_tensor("RbA", [64, NHP, CH], BF16).ap()
    RbB = nc.alloc_sbuf_tensor("RbB", [64, NHP, CH], BF16).ap()
    kcache = nc.alloc_sbuf_tensor("kcache", [P, NCHUNK, NG, CH], BF16).ap()
    vcache = nc.alloc_sbuf_tensor("vcache", [P, NCHUNK, NT, H, DH + 1], BF16).ap()
    mdense_s = nc.alloc_sbuf_tensor("mdense_s", [P, NCHUNK], F32).ap()
    padcol_s = nc.alloc_sbuf_tensor("padcol_s", [P, NT], F32).ap()
    padfull_s = nc.alloc_sbuf_tensor("padfull_s", [P, NCHUNK, NT], F32).ap()
    bqk_s = nc.alloc_sbuf_tensor("bqk_s", [P, L, 16], F32).ap()
    b1c_s = nc.alloc_sbuf_tensor("b1c_s", [P, L, NF], F32).ap()
    trim_s = nc.alloc_sbuf_tensor("trim_s", [P, NT, CH], BF16).ap()
    ident_s = nc.alloc_sbuf_tensor("ident_s", [P, P], BF16).ap()
    eps_s = nc.alloc_sbuf_tensor("eps_s", [P, 1], F32).ap()
    stats = nc.alloc_sbuf_tensor("stats", [P, 6, NT], F32).ap()  # sum,sq,mu,mu2,var,rs
    if with_bias:
        bias_bc = nc.alloc_sbuf_tensor("bias_bc", [P, 3, D], BF16).ap()

    with tile.TileContext(nc) as tc:
        with (
            tc.tile_pool(name="big", bufs=4) as big,       # qT/kTloc/vloc/gT overlay
            tc.tile_pool(name="wA", bufs=2) as wA,         # [P, NG, 512] weights
            tc.tile_pool(name="wB", bufs=4) as wB,         # [P, 512] w2 tiles
            tc.tile_pool(name="htm", bufs=2) as htmp,      # token-major h staging
            tc.tile_pool(name="attn", bufs=3) as attnp,    # exp outputs
            tc.tile_pool(name="lrec", bufs=4) as lrecp,
            tc.tile_pool(name="lrecf", bufs=2) as lrecf,    # softmax denom recip
            tc.tile_pool(name="sc", bufs=2, space="PSUM") as scp,    # 2 banks/slot
            tc.tile_pool(name="oacc", bufs=2, space="PSUM") as oaccp,  # 1 bank
            tc.tile_pool(name="mm", bufs=2, space="PSUM") as mmp,      # 1 bank
        ):
            # one-time loads
            nc.vector.memset(eps_s, 1e-5)
            nc.sync.dma_start(xs, x0.rearrange("(tt p) d -> p tt d", p=P))
            nc.sync.dma_start(mdense_s, mdense[:])
            nc.sync.dma_start(padcol_s, padcol[:])
            nc.sync.dma_start(padfull_s, padfull[:])
            nc.sync.dma_start(bqk_s, bqk.rearrange("l p x -> p l x"))
            nc.sync.dma_start(b1c_s, b1c.rearrange("l p x -> p l x"))
            nc.sync.dma_start(trim_s, trim.rearrange("j p t -> p j t"))
            nc.sync.dma_start(ident_s, ident[:])

            def ln_to_hT(lix):
                """LayerNorm(xs) -> token-major bf16 -> transpose into hT."""
                ssum = stats[:, 0]
                ssq = stats[:, 1]
                mu = stats[:, 2]
                mu2 = stats[:, 3]
                var = stats[:, 4]
                rs = stats[:, 5]
                htiles = []
                for tt in range(NT):
                    nc.vector.reduce_sum(
                        out=ssum[:, tt : tt + 1], in_=xs[:, tt], axis=AX.X
                    )
                    scr = htmp.tile([P, D], BF16, tag="htm")
                    nc.scalar.activation(
                        scr, xs[:, tt], AF.Square, accum_out=ssq[:, tt : tt + 1]
                    )
                nc.vector.tensor_scalar_mul(mu, ssum, 1.0 / D)
                nc.vector.tensor_mul(out=mu2, in0=mu, in1=mu)
                nc.vector.tensor_scalar_mul(var, ssq, 1.0 / D)
                nc.vector.tensor_sub(out=var, in0=var, in1=mu2)
                # rs = 1/sqrt(var + eps); DVE copy in between keeps the
                # Reciprocal custom-op at a single wait slot (walrus limit)
                nc.scalar.activation(var, var, AF.Sqrt, bias=eps_s[:, 0:1])
                nc.vector.tensor_copy(out=mu2, in_=var)
                nc.vector.reciprocal(rs, mu2)
                for tt in range(NT):
                    htile = htmp.tile([P, D], BF16, tag="htm")
                    nc.vector.tensor_scalar(
                        htile,
                        xs[:, tt],
                        mu[:, tt : tt + 1],
                        rs[:, tt : tt + 1],
                        mybir.AluOpType.subtract,
                        mybir.AluOpType.mult,
                    )
                    htiles.append(htile)
                    for g in range(NG):
                        pt = mmp.tile([P, P], BF16, tag="mm")
                        nc.tensor.transpose(pt, htile[:, ts(g, P)], ident_s)
                        nc.vector.tensor_copy(out=hT[:, g, ts(tt, P)], in_=pt)

            def qkv(l, qT, kTloc, vloc):
                # Q^T and K^T (feature-major)
                for which, wmat, dst, bcol in (
                    (0, wq, qT, 0),
                    (1, wk, kTloc, 8),
                ):
                    for g in range(NG):
                        wt = wA.tile([P, NG, P], BF16, tag="wA")
                        nc.sync.dma_start(
                            wt,
                            wmat[l, :, ts(g, P)].rearrange(
                                "(kd p) f -> p kd f", p=P
                            ),
                        )
                        ps = mmp.tile([P, CH], F32, tag="mm")
                        for kd in range(NG):
                            nc.tensor.matmul(
                                ps,
                                lhsT=wt[:, kd],
                                rhs=hT[:, kd],
                                start=(kd == 0),
                                stop=(kd == NG - 1),
                            )
                        nc.vector.tensor_scalar_add(
                            dst[:, g], ps, bqk_s[:, l, bcol + g : bcol + g + 1]
                        )
                # V (token-major, padded rows zeroed, into aug layout)
                for n in range(2):
                    wvt = wA.tile([P, NG, CH], BF16, tag="wA")
                    nc.sync.dma_start(
                        wvt,
                        wv[l, :, ts(n, CH)].rearrange("(kd p) f -> p kd f", p=P),
                    )
                    for tt in range(NT):
                        ps = mmp.tile([P, CH], F32, tag="mm")
                        for kd in range(NG):
                            nc.tensor.matmul(
                                ps,
                                lhsT=hT[:, kd, ts(tt, P)],
                                rhs=wvt[:, kd],
                                start=(kd == 0),
                                stop=(kd == NG - 1),
                            )
                        if with_bias:
                            nc.vector.tensor_tensor(
                                ps, ps, bias_bc[:, 0, ts(n, CH)], mybir.AluOpType.add
                            )
                        nc.vector.tensor_scalar_mul(
                            vloc[:, tt, 8 * n : 8 * (n + 1), 0:DH],
                            ps.rearrange("p (h e) -> p h e", h=8),
                            padcol_s[:, tt : tt + 1],
                        )
                # ones column = padcol (zero for invalid keys)
                nc.vector.tensor_copy(
                    out=vloc[:, :, :, DH : DH + 1],
                    in_=padcol_s[:, :, None, None].to_broadcast([P, NT, H, 1]),
                )

            def allgather(l, kTloc, vloc):
                nc.sync.dma_start(
                    kvin[l][0].rearrange("p (g t) -> p g t", g=NG), kTloc
                )
                for tt in range(NT):
                    nc.sync.dma_start(
                        kvin[l][1, :, tt * H * DH : (tt + 1) * H * DH].rearrange(
                            "p (h e) -> p h e", h=H
                        ),
                        vloc[:, tt, :, 0:DH],
                    )
                if for_sim:
                    # stand-in for the collective so TimelineSim (which
                    # rejects collectives) keeps the dependency edge
                    nc.sync.dma_start(kvall[l][0], kvin[l][:])
                else:
                    nc.gpsimd.collective_compute(
                        "AllGather",
                        mybir.AluOpType.bypass,
                        replica_groups=[[0, 1, 2, 3], [4, 5, 6, 7]],
                        ins=[kvin[l][:]],
                        outs=[kvall[l][:]],
                    )
                nc.sync.dma_start(
                    kcache,
                    kvall[l][:, 0].rearrange("c p (g t) -> p c g t", g=NG),
                )
                for c in range(NCHUNK):
                    for tt in range(NT):
                        nc.sync.dma_start(
                            vcache[:, c, tt, :, 0:DH],
                            kvall[l][
                                c, 1, :, tt * H * DH : (tt + 1) * H * DH
                            ].rearrange("p (h e) -> p h e", h=H),
                        )
                nc.vector.tensor_copy(
                    out=vcache[:, :, :, :, DH : DH + 1],
                    in_=padfull_s[:, :, :, None, None].to_broadcast(
                        [P, NCHUNK, NT, H, 1]
                    ),
                )

            def attention(qT, kTloc, vloc):
                for hp in range(NHP):
                    oa = oaccp.tile([P, CH], F32, tag="oacc")
                    ob = oaccp.tile([P, CH], F32, tag="oacc")
                    # (lhsT_a, lhsT_b, vlhsT_a, vlhsT_b, bias, trimul)
                    steps = []
                    for c in range(NCHUNK):
                        for s in range(NT):
                            steps.append(
                                (
                                    kcache[0:64, c, hp, ts(s, P)],
                                    kcache[64:P, c, hp, ts(s, P)],
                                    vcache[:, c, s, 2 * hp],
                                    vcache[:, c, s, 2 * hp + 1],
                                    mdense_s[:, c : c + 1],
                                    None,
                                )
                            )
                    for j in range(NT):
                        steps.append(
                            (
                                kTloc[0:64, hp, ts(j, P)],
                                kTloc[64:P, hp, ts(j, P)],
                                vloc[:, j, 2 * hp],
                                vloc[:, j, 2 * hp + 1],
                                0.0,
                                trim_s[:, j],
                            )
                        )
                    nsteps = len(steps)
                    for i, (ka, kb, va, vb, bias, tmask) in enumerate(steps):
                        sc = scp.tile([P, 2, CH], F32, tag="sc")
                        nc.tensor.matmul(
                            sc[:, 0], lhsT=ka, rhs=qT[0:64, hp], start=True, stop=True
                        )
                        nc.tensor.matmul(
                            sc[:, 1], lhsT=kb, rhs=qT[64:P, hp], start=True, stop=True
                        )
                        at = attnp.tile([P, 2, CH], BF16, tag="attn")
                        nc.scalar.activation(at, sc, AF.Exp, bias=bias, scale=SCALE)
                        if tmask is not None:
                            nc.vector.tensor_mul(out=at[:, 0], in0=at[:, 0], in1=tmask)
                            nc.vector.tensor_mul(out=at[:, 1], in0=at[:, 1], in1=tmask)
                        nc.tensor.matmul(
                            oa[0 : DH + 1],
                            lhsT=va,
                            rhs=at[:, 0],
                            start=(i == 0),
                            stop=(i == nsteps - 1),
                        )
                        nc.tensor.matmul(
                            ob[0 : DH + 1],
                            lhsT=vb,
                            rhs=at[:, 1],
                            start=(i == 0),
                            stop=(i == nsteps - 1),
                        )
                    # extract o rows and denominators
                    nc.vector.tensor_copy(out=oT[0:64, hp], in_=oa[0:64])
                    nc.vector.tensor_copy(out=oTb[:, hp], in_=ob[0:64])
                    # move the denominator rows (psum partition 64) to
                    # partition 0 via DMA, reciprocal there, then broadcast
                    ls = lrecf.tile([DH + 1, 2, CH], BF16, tag="lrecf")
                    nc.vector.tensor_copy(out=ls[64:65, 0], in_=oa[64:65])
                    nc.vector.tensor_copy(out=ls[64:65, 1], in_=ob[64:65])
                    la = lrecf.tile([1, 2, CH], BF16, tag="lrecf2")
                    nc.sync.dma_start(la, ls[64:65])
                    lb = la[:, 1]
                    la = la[:, 0]
                    ra = lrecp.tile([1, CH], BF16, tag="lrec")
                    rb = lrecp.tile([1, CH], BF16, tag="lrec")
                    with nc.allow_low_precision(
                        reason="softmax denom reciprocal in bf16 is within tolerance"
                    ):
                        nc.vector.reciprocal(ra, la)
                        nc.vector.reciprocal(rb, lb)
                    nc.gpsimd.partition_broadcast(RbA[:, hp], ra)
                    nc.gpsimd.partition_broadcast(RbB[:, hp], rb)
                # normalize each parity at partition-0 alignment, then merge
                nc.vector.tensor_mul(out=oT[0:64], in0=oT[0:64], in1=RbA)
                nc.vector.tensor_mul(out=oTb, in0=oTb, in1=RbB)
                nc.sync.dma_start(oT[64:P], oTb)

            def outproj(l):
                for n in range(2):
                    wot = wA.tile([P, NG, CH], BF16, tag="wA")
                    nc.sync.dma_start(
                        wot,
                        wo[l, :, ts(n, CH)].rearrange("(g p) f -> p g f", p=P),
                    )
                    for m in range(NT):
                        ps = mmp.tile([P, CH], F32, tag="mm")
                        for g in range(NG):
                            nc.tensor.matmul(
                                ps,
                                lhsT=oT[:, g, ts(m, P)],
                                rhs=wot[:, g],
                                start=(g == 0),
                                stop=(g == NG - 1),
                            )
                        if with_bias:
                            nc.vector.tensor_tensor(
                                ps, ps, bias_bc[:, 1, ts(n, CH)], mybir.AluOpType.add
                            )
                        nc.vector.tensor_add(
                            out=xs[:, m, ts(n, CH)], in0=xs[:, m, ts(n, CH)], in1=ps
                        )

            def ffn(l, gts):
                for fs in range(NF):
                    w1t = wA.tile([P, NG, P], BF16, tag="wA")
                    nc.sync.dma_start(
                        w1t,
                        w1[l, :, ts(fs, P)].rearrange("(kd p) f -> p kd f", p=P),
                    )
                    ps = mmp.tile([P, CH], F32, tag="mm")
                    for kd in range(NG):
                        nc.tensor.matmul(
                            ps,
                            lhsT=w1t[:, kd],
                            rhs=hT[:, kd],
                            start=(kd == 0),
                            stop=(kd == NG - 1),
                        )
                    nc.scalar.activation(
                        gts[fs // NG][:, fs % NG],
                        ps,
                        AF.Gelu,
                        bias=b1c_s[:, l, fs : fs + 1],
                    )
                for n in range(2):
                    psA = scp.tile([P, 2, CH], F32, tag="sc")
                    psB = scp.tile([P, 2, CH], F32, tag="sc")
                    pslices = [psA[:, 0], psA[:, 1], psB[:, 0], psB[:, 1]]
                    for fs in range(NF):
                        w2t = wB.tile([P, CH], BF16, tag="wB")
                        nc.sync.dma_start(w2t, w2[l, ts(fs, P), ts(n, CH)])
                        for m in range(NT):
                            nc.tensor.matmul(
                                pslices[m],
                                lhsT=gts[fs // NG][:, fs % NG, ts(m, P)],
                                rhs=w2t,
                                start=(fs == 0),
                                stop=(fs == NF - 1),
                            )
                    for m in range(NT):
                        if with_bias:
                            nc.vector.tensor_tensor(
                                pslices[m],
                                pslices[m],
                                bias_bc[:, 2, ts(n, CH)],
                                mybir.AluOpType.add,
                            )
                        nc.vector.tensor_add(
                            out=xs[:, m, ts(n, CH)],
                            in0=xs[:, m, ts(n, CH)],
                            in1=pslices[m],
                        )

            for l in range(nlayers):
                if with_bias:
                    nc.gpsimd.dma_start(
                        bias_bc,
                        brows[None, :, :].to_broadcast([P, 3, D]),
                    )
                qT = big.tile([P, NHP, CH], BF16, tag="big")
                kTloc = big.tile([P, NG, CH], BF16, tag="big")
                vloc = big.tile([P, NT, H, DH + 1], BF16, tag="big")
                ln_to_hT(l)
                if dump and l == 0:
                    nc.sync.dma_start(dbg_hT[:], hT)
                qkv(l, qT, kTloc, vloc)
                if dump and l == 0:
                    nc.sync.dma_start(dbg_qT[:], qT)
                    nc.sync.dma_start(dbg_kT[:], kTloc)
                    nc.sync.dma_start(dbg_vloc[:], vloc)
                allgather(l, kTloc, vloc)
                if dump and l == 0:
                    nc.sync.dma_start(dbg_kc[:], kcache)
                    nc.sync.dma_start(dbg_vc[:], vcache)
                attention(qT, kTloc, vloc)
                if dump and l == 0:
                    nc.sync.dma_start(dbg_oT[:], oT)
                    nc.sync.dma_start(dbg_Rb[0:64], RbA)
                    nc.sync.dma_start(dbg_Rb[64:P], RbB)
                outproj(l)
                if dump and l == 0:
                    nc.sync.dma_start(dbg_x1[:], xs)
                ln_to_hT(l)
                gts = [
                    big.tile([P, NG, CH], BF16, tag="big", name=f"gts{i}")
                    for i in range(4)
                ]
                ffn(l, gts)

            nc.sync.dma_start(y.rearrange("(tt p) d -> p tt d", p=P), xs)

    nc.finalize()
    return nc


# ------------------------- host side -------------------------


def _layer_norm_np(x):
    mu = x.mean(-1, keepdims=True)
    var = ((x - mu) ** 2).mean(-1, keepdims=True)
    return (x - mu) / np.sqrt(var + 1e-5)


def _sinusoidal_pe():
    pos = np.arange(T, dtype=np.float32)[:, None]
    div = np.exp(np.arange(0, D, 2, dtype=np.float32) * (-np.log(10000.0) / D))
    pe = np.zeros((T, D), dtype=np.float32)
    pe[:, 0::2] = np.sin(pos * div)
    pe[:, 1::2] = np.cos(pos * div)
    return pe


def kernel(**inputs) -> np.ndarray:
    x = np.asarray(inputs["x"], np.float32)
    lengths = np.asarray(inputs["lengths"]).astype(np.int64)
    Wqkv = np.asarray(inputs["Wqkv"], np.float32)
    bqkv = np.asarray(inputs["bqkv"], np.float32)
    Wo = np.asarray(inputs["Wo"], np.float32)
    bo = np.asarray(inputs["bo"], np.float32)
    ln0_g = np.asarray(inputs["ln0_g"], np.float32)
    ln0_b = np.asarray(inputs["ln0_b"], np.float32)
    ln1_g = np.asarray(inputs["ln1_g"], np.float32)
    ln1_b = np.asarray(inputs["ln1_b"], np.float32)
    ln2_g = np.asarray(inputs["ln2_g"], np.float32)
    ln2_b = np.asarray(inputs["ln2_b"], np.float32)
    W1 = np.asarray(inputs["W1"], np.float32)
    b1 = np.asarray(inputs["b1"], np.float32)
    W2 = np.asarray(inputs["W2"], np.float32)
    b2 = np.asarray(inputs["b2"], np.float32)

    bf16 = ml_dtypes.bfloat16

    # LN0 + positional encoding on host
    x0 = _layer_norm_np(x) * ln0_g + ln0_b + _sinusoidal_pe()[None]
    x0 = x0.astype(np.float32)

    # fold ln1/ln2 affine into the first matmul of each block
    Wqkv_eff = ln1_g[:, :, None] * Wqkv
    bqkv_eff = bqkv + np.einsum("ld,ldn->ln", ln1_b, Wqkv)
    W1_eff = ln2_g[:, :, None] * W1
    b1_eff = b1 + np.einsum("ld,ldn->ln", ln2_b, W1)

    wq_h = np.ascontiguousarray(Wqkv_eff[:, :, 0:D]).astype(bf16)
    wk_h = np.ascontiguousarray(Wqkv_eff[:, :, D : 2 * D]).astype(bf16)
    wv_h = np.ascontiguousarray(Wqkv_eff[:, :, 2 * D : 3 * D]).astype(bf16)
    wo_h = Wo.astype(bf16)
    w1_h = W1_eff.astype(bf16)
    w2_h = W2.astype(bf16)

    bq_eff = bqkv_eff[:, 0:D]
    bk_eff = bqkv_eff[:, D : 2 * D]
    bv_eff = bqkv_eff[:, 2 * D : 3 * D]

    # per-partition bias columns for Q/K: [L, P, 16] (cols 0:8 = q, 8:16 = k)
    bqk_h = np.zeros((L, P, 16), np.float32)
    for g in range(NG):
        bqk_h[:, :, g] = bq_eff[:, g * P : (g + 1) * P]
        bqk_h[:, :, 8 + g] = bk_eff[:, g * P : (g + 1) * P]
    b1c_h = np.zeros((L, P, NF), np.float32)
    for fs in range(NF):
        b1c_h[:, :, fs] = b1_eff[:, fs * P : (fs + 1) * P]

    with_bias = not (
        np.all(bv_eff == 0.0) and np.all(bo == 0.0) and np.all(b2 == 0.0)
    )
    brows_h = np.stack([bv_eff.sum(0) * 0, bo.sum(0) * 0, b2.sum(0) * 0]).astype(bf16)
    if with_bias:
        # biases are per-layer; the kernel adds the same row each layer, so the
        # general path is only valid when rows are layer-independent.
        same = (
            np.all(bv_eff == bv_eff[0]) and np.all(bo == bo[0]) and np.all(b2 == b2[0])
        )
        if not same:
            raise NotImplementedError("per-layer V/O/FFN2 biases not supported")
        brows_h = np.stack([bv_eff[0], bo[0], b2[0]]).astype(bf16)

    # static triangular masks per diagonal j-tile: [NT, P, CH]
    trim_h = np.zeros((NT, P, CH), np.float32)
    for j in range(NT):
        for i in range(NT):
            blk = trim_h[j][:, i * P : (i + 1) * P]
            if j < i:
                blk[:] = 1.0
            elif j == i:
                blk[:] = np.tril(np.ones((P, P), np.float32)).T  # keep tk <= tq
    trim_h = trim_h.astype(bf16)
    ident_h = np.eye(P, dtype=np.float32).astype(bf16)

    key = with_bias
    if key not in _CACHE:
        _CACHE[key] = _build(with_bias)
    nc = _CACHE[key]

    in_maps = []
    for core in range(8):
        b, q = core // 4, core % 4
        pos = q * CH + np.arange(CH)
        valid = (pos < lengths[b]).astype(np.float32)  # [CH]
        padcol_h = valid.reshape(NT, P).T.copy()  # [P, NT]
        posf = np.arange(T)
        validf = (posf < lengths[b]).astype(np.float32)
        padfull_h = validf.reshape(NCHUNK, NT, P).transpose(2, 0, 1).copy()
        mdense_h = np.zeros((P, NCHUNK), np.float32)
        for c in range(NCHUNK):
            if c >= q:
                mdense_h[:, c] = NEG
        m = {
            "x0": np.ascontiguousarray(x0[b, q * CH : (q + 1) * CH]),
            "wq": wq_h,
            "wk": wk_h,
            "wv": wv_h,
            "wo": wo_h,
            "w1": w1_h,
            "w2": w2_h,
            "bqk": bqk_h,
            "b1c": b1c_h,
            "mdense": mdense_h,
            "padcol": padcol_h,
            "padfull": padfull_h,
            "trim": np.ascontiguousarray(trim_h),
            "ident": ident_h,
        }
        if with_bias:
            m["brows"] = brows_h
        in_maps.append(m)

    res = run_bass_kernel_spmd(
        nc,
        in_maps,
        core_ids=list(range(8)),
        trace=bool(os.environ.get("KERNEL_TRACE")),
    )
    globals()["LAST_RESULT"] = res
    out = np.zeros((B, T, D), np.float32)
    for core in range(8):
        b, q = core // 4, core % 4
        out[b, q * CH : (q + 1) * CH] = res.results[core]["y"]
    return out



# revision 5
# speedup vs baseline: 1.1676x; 1.1676x over previous
"""Trainium2 Bass kernel for nn_CausalTransformerEncoder.

Sharding: 8 cores = 2 (batch) x 4 (sequence chunks of 512 tokens).
Per layer: each core computes LN + K/V for its own 512 tokens, launches the
K/V AllGather within its 4-core batch group, then computes Q and runs the
local-diagonal attention steps (static triangular masks) while the
collective is in flight.  Once the gathered K/V land, it runs attention
over the 3 earlier chunks (block-causal via per-core bias tables; the
fully-masked later chunks are never computed), then out-proj, LN2 and FFN
on its own tokens.  Residual stream stays fp32 in SBUF; matmuls run bf16.

Softmax: exp without max subtraction (scores are small; validated on host),
denominator for free via a ones-column appended to V (matmul M=65 outputs
o rows 0:64 and the sum at row 64). Key-padding from `lengths` is handled
by zeroing V rows + ones-column at the source, so masked keys drop out of
both numerator and denominator with no extra masking work.
"""

import sys
import os

for _p in ("/opt/trn_rl_repo", os.path.expanduser("~/.axon_site/_ro/trn_rl_repo")):
    if os.path.isdir(_p) and _p not in sys.path:
        sys.path.insert(0, _p)

import numpy as np
import ml_dtypes

import concourse.bass as bass
from concourse import bacc
import concourse.mybir as mybir
import concourse.tile as tile
from concourse.bass import ts
from concourse.bass_utils import run_bass_kernel_spmd

F32 = mybir.dt.float32
BF16 = mybir.dt.bfloat16
AF = mybir.ActivationFunctionType
AX = mybir.AxisListType

B, T, D, H, L, FF = 2, 2048, 1024, 16, 4, 4096
DH = D // H          # 64
P = 128
CH = 512             # tokens per core
NT = CH // P         # 4 token tiles per core
NG = D // P          # 8 feature sub-tiles
NF = FF // P         # 32 ffn sub-tiles
NHP = H // 2         # 8 head pairs
NCHUNK = 4           # sequence chunks per batch group
NCG = 3              # gathered chunk slots kept for attention (c < own q)
SCALE = 1.0 / np.sqrt(DH)
NEG = -1.0e9

_CACHE = {}


def _build(with_bias: bool, nlayers: int = L, for_sim: bool = False):
    nc = bacc.Bacc("TRN2", target_bir_lowering=False, debug=False, num_devices=8)

    # ---------------- I/O ----------------
    x0 = nc.dram_tensor("x0", [CH, D], F32, kind="ExternalInput")
    wq = nc.dram_tensor("wq", [L, D, D], BF16, kind="ExternalInput")
    wk = nc.dram_tensor("wk", [L, D, D], BF16, kind="ExternalInput")
    wv = nc.dram_tensor("wv", [L, D, D], BF16, kind="ExternalInput")
    wo = nc.dram_tensor("wo", [L, D, D], BF16, kind="ExternalInput")
    w1 = nc.dram_tensor("w1", [L, D, FF], BF16, kind="ExternalInput")
    w2 = nc.dram_tensor("w2", [L, FF, D], BF16, kind="ExternalInput")
    bqk = nc.dram_tensor("bqk", [L, P, 16], F32, kind="ExternalInput")
    b1c = nc.dram_tensor("b1c", [L, P, NF], F32, kind="ExternalInput")
    mdense = nc.dram_tensor("mdense", [P, NCG], F32, kind="ExternalInput")
    padcol = nc.dram_tensor("padcol", [P, NT], F32, kind="ExternalInput")
    padfull = nc.dram_tensor("padfull", [P, NCG, NT], F32, kind="ExternalInput")
    trim = nc.dram_tensor("trim", [NT, P, CH], BF16, kind="ExternalInput")
    ident = nc.dram_tensor("ident", [P, P], BF16, kind="ExternalInput")
    if with_bias:
        brows = nc.dram_tensor("brows", [3, D], BF16, kind="ExternalInput")
    y = nc.dram_tensor("y", [CH, D], F32, kind="ExternalOutput")

    # per-layer DRAM for the collective
    kvin = [nc.dram_tensor(f"kvin{l}", [2, P, NG * CH], BF16) for l in range(L)]
    kvall = [
        nc.dram_tensor(f"kvall{l}", [NCHUNK, 2, P, NG * CH], BF16)
        for l in range(L)
    ]

    # ---------------- persistent SBUF ----------------
    xs = nc.alloc_sbuf_tensor("xs", [P, NT, D], F32).ap()
    hT = nc.alloc_sbuf_tensor("hT", [P, NG, CH], BF16).ap()
    oT = nc.alloc_sbuf_tensor("oT", [P, NHP, CH], BF16).ap()
    oTb = nc.alloc_sbuf_tensor("oTb", [64, NHP, CH], BF16).ap()
    olocA = nc.alloc_sbuf_tensor("olocA", [DH + 1, NHP, CH], F32).ap()
    olocB = nc.alloc_sbuf_tensor("olocB", [DH + 1, NHP, CH], F32).ap()
    kcache = nc.alloc_sbuf_tensor("kcache", [P, NCG, NG, CH], BF16).ap()
    vcache = nc.alloc_sbuf_tensor("vcache", [P, NCG, NT, H, DH + 1], BF16).ap()
    mdense_s = nc.alloc_sbuf_tensor("mdense_s", [P, NCG], F32).ap()
    padcol_s = nc.alloc_sbuf_tensor("padcol_s", [P, NT], F32).ap()
    padfull_s = nc.alloc_sbuf_tensor("padfull_s", [P, NCG, NT], F32).ap()
    bqk_s = nc.alloc_sbuf_tensor("bqk_s", [P, L, 16], F32).ap()
    b1c_s = nc.alloc_sbuf_tensor("b1c_s", [P, L, NF], F32).ap()
    trim_s = nc.alloc_sbuf_tensor("trim_s", [P, NT, CH], BF16).ap()
    ident_s = nc.alloc_sbuf_tensor("ident_s", [P, P], BF16).ap()
    eps_s = nc.alloc_sbuf_tensor("eps_s", [P, 1], F32).ap()
    stats = nc.alloc_sbuf_tensor("stats", [P, 6, NT], F32).ap()  # sum,sq,mu,mu2,var,rs
    if with_bias:
        bias_bc = nc.alloc_sbuf_tensor("bias_bc", [P, 3, D], BF16).ap()

    with tile.TileContext(nc) as tc:
        with (
            tc.tile_pool(name="big", bufs=4) as big,       # qT/kTloc/vloc/gT overlay
            tc.tile_pool(name="wA", bufs=2) as wA,         # [P, NG, 512] weights
            tc.tile_pool(name="wB", bufs=3) as wB,         # [P, 512] w2 tiles
            tc.tile_pool(name="htm", bufs=2) as htmp,      # token-major h staging
            tc.tile_pool(name="attn", bufs=3) as attnp,    # exp outputs
            tc.tile_pool(name="lrec", bufs=2) as lrecp,
            tc.tile_pool(name="lrecf", bufs=1) as lrecf,    # softmax denom recip
            tc.tile_pool(name="sc", bufs=3, space="PSUM") as scp,    # 2 banks/slot
            tc.tile_pool(name="oacc", bufs=2, space="PSUM") as oaccp,  # 1 bank
        ):
            # one-time loads
            nc.vector.memset(eps_s, 1e-5)
            nc.sync.dma_start(xs, x0.rearrange("(tt p) d -> p tt d", p=P))
            nc.sync.dma_start(mdense_s, mdense[:])
            nc.sync.dma_start(padcol_s, padcol[:])
            nc.sync.dma_start(padfull_s, padfull[:])
            nc.sync.dma_start(bqk_s, bqk.rearrange("l p x -> p l x"))
            nc.sync.dma_start(b1c_s, b1c.rearrange("l p x -> p l x"))
            nc.sync.dma_start(trim_s, trim.rearrange("j p t -> p j t"))
            nc.sync.dma_start(ident_s, ident[:])

            _mm_ctr = [0]

            def mm_ps(shape, dtype):
                # [P, CH]-or-smaller PSUM scratch shares the score pool's
                # 2-bank slots (the score tiles are live only inside
                # attention, these only outside it)
                _mm_ctr[0] += 1
                return scp.tile(shape, dtype, tag="sc", name=f"mmps{_mm_ctr[0]}")

            def ln_to_hT(lix):
                """LayerNorm(xs) -> token-major bf16 -> transpose into hT."""
                ssum = stats[:, 0]
                ssq = stats[:, 1]
                mu = stats[:, 2]
                mu2 = stats[:, 3]
                var = stats[:, 4]
                rs = stats[:, 5]
                htiles = []
                for tt in range(NT):
                    nc.vector.reduce_sum(
                        out=ssum[:, tt : tt + 1], in_=xs[:, tt], axis=AX.X
                    )
                    scr = htmp.tile([P, D], BF16, tag="htm")
                    nc.scalar.activation(
                        scr, xs[:, tt], AF.Square, accum_out=ssq[:, tt : tt + 1]
                    )
                nc.vector.tensor_scalar_mul(mu, ssum, 1.0 / D)
                nc.vector.tensor_mul(out=mu2, in0=mu, in1=mu)
                nc.vector.tensor_scalar_mul(var, ssq, 1.0 / D)
                nc.vector.tensor_sub(out=var, in0=var, in1=mu2)
                # rs = 1/sqrt(var + eps); DVE copy in between keeps the
                # custom-op at a single wait slot (walrus limit)
                nc.scalar.activation(var, var, AF.Sqrt, bias=eps_s[:, 0:1])
                nc.vector.tensor_copy(out=mu2, in_=var)
                nc.vector.reciprocal_approx_fast(rs, mu2)
                for tt in range(NT):
                    htile = htmp.tile([P, D], BF16, tag="htm")
                    nc.vector.tensor_scalar(
                        htile,
                        xs[:, tt],
                        mu[:, tt : tt + 1],
                        rs[:, tt : tt + 1],
                        mybir.AluOpType.subtract,
                        mybir.AluOpType.mult,
                    )
                    htiles.append(htile)
                    for g in range(NG):
                        pt = mm_ps([P, P], BF16)
                        nc.tensor.transpose(pt, htile[:, ts(g, P)], ident_s)
                        nc.vector.tensor_copy(out=hT[:, g, ts(tt, P)], in_=pt)

            def kv_proj(l, kTloc, vloc):
                # K^T (feature-major)
                for g in range(NG):
                    wt = wA.tile([P, NG, P], BF16, tag="wA")
                    nc.sync.dma_start(
                        wt,
                        wk[l, :, ts(g, P)].rearrange("(kd p) f -> p kd f", p=P),
                    )
                    ps = mm_ps([P, CH], F32)
                    for kd in range(NG):
                        nc.tensor.matmul(
                            ps,
                            lhsT=wt[:, kd],
                            rhs=hT[:, kd],
                            start=(kd == 0),
                            stop=(kd == NG - 1),
                        )
                    nc.vector.tensor_scalar_add(
                        kTloc[:, g], ps, bqk_s[:, l, 8 + g : 8 + g + 1]
                    )
                # V (token-major, padded rows zeroed, into aug layout)
                for n in range(2):
                    wvt = wA.tile([P, NG, CH], BF16, tag="wA")
                    nc.sync.dma_start(
                        wvt,
                        wv[l, :, ts(n, CH)].rearrange("(kd p) f -> p kd f", p=P),
                    )
                    for tt in range(NT):
                        ps = mm_ps([P, CH], F32)
                        for kd in range(NG):
                            nc.tensor.matmul(
                                ps,
                                lhsT=hT[:, kd, ts(tt, P)],
                                rhs=wvt[:, kd],
                                start=(kd == 0),
                                stop=(kd == NG - 1),
                            )
                        if with_bias:
                            nc.vector.tensor_tensor(
                                ps, ps, bias_bc[:, 0, ts(n, CH)], mybir.AluOpType.add
                            )
                        nc.vector.tensor_scalar_mul(
                            vloc[:, tt, 8 * n : 8 * (n + 1), 0:DH],
                            ps.rearrange("p (h e) -> p h e", h=8),
                            padcol_s[:, tt : tt + 1],
                        )
                # ones column = padcol (zero for invalid keys)
                nc.vector.tensor_copy(
                    out=vloc[:, :, :, DH : DH + 1],
                    in_=padcol_s[:, :, None, None].to_broadcast([P, NT, H, 1]),
                )

            def q_proj(l, qT):
                for g in range(NG):
                    wt = wA.tile([P, NG, P], BF16, tag="wA")
                    nc.sync.dma_start(
                        wt,
                        wq[l, :, ts(g, P)].rearrange("(kd p) f -> p kd f", p=P),
                    )
                    ps = mm_ps([P, CH], F32)
                    for kd in range(NG):
                        nc.tensor.matmul(
                            ps,
                            lhsT=wt[:, kd],
                            rhs=hT[:, kd],
                            start=(kd == 0),
                            stop=(kd == NG - 1),
                        )
                    nc.vector.tensor_scalar_add(
                        qT[:, g], ps, bqk_s[:, l, g : g + 1]
                    )

            def allgather_launch(l, kTloc, vloc):
                nc.sync.dma_start(
                    kvin[l][0].rearrange("p (g t) -> p g t", g=NG), kTloc
                )
                for tt in range(NT):
                    nc.sync.dma_start(
                        kvin[l][1, :, tt * H * DH : (tt + 1) * H * DH].rearrange(
                            "p (h e) -> p h e", h=H
                        ),
                        vloc[:, tt, :, 0:DH],
                    )
                if for_sim:
                    # stand-in for the collective so TimelineSim (which
                    # rejects collectives) keeps the dependency edge
                    nc.sync.dma_start(kvall[l][0], kvin[l][:])
                else:
                    nc.gpsimd.collective_compute(
                        "AllGather",
                        mybir.AluOpType.bypass,
                        replica_groups=[[0, 1, 2, 3], [4, 5, 6, 7]],
                        ins=[kvin[l][:]],
                        outs=[kvall[l][:]],
                    )

            def gather_land(l):
                # per-chunk DMAs so chunk-0 attention can start first
                for c in range(NCG):
                    nc.sync.dma_start(
                        kcache[:, c],
                        kvall[l][c, 0].rearrange("p (g t) -> p g t", g=NG),
                    )
                    for tt in range(NT):
                        nc.sync.dma_start(
                            vcache[:, c, tt, :, 0:DH],
                            kvall[l][
                                c, 1, :, tt * H * DH : (tt + 1) * H * DH
                            ].rearrange("p (h e) -> p h e", h=H),
                        )
                    nc.vector.tensor_copy(
                        out=vcache[:, c, :, :, DH : DH + 1],
                        in_=padfull_s[:, c, :, None, None].to_broadcast(
                            [P, NT, H, 1]
                        ),
                    )

            def attn_step(hp, qT, ka, kb, va, vb, bias, tmask, oa, ob, start, stop):
                sc = scp.tile([P, 2, CH], F32, tag="sc")
                nc.tensor.matmul(
                    sc[:, 0], lhsT=ka, rhs=qT[0:64, hp], start=True, stop=True
                )
                nc.tensor.matmul(
                    sc[:, 1], lhsT=kb, rhs=qT[64:P, hp], start=True, stop=True
                )
                at = attnp.tile([P, 2, CH], BF16, tag="attn")
                nc.scalar.activation(at, sc, AF.Exp, bias=bias, scale=SCALE)
                if tmask is not None:
                    nc.vector.tensor_mul(out=at[:, 0], in0=at[:, 0], in1=tmask)
                    nc.vector.tensor_mul(out=at[:, 1], in0=at[:, 1], in1=tmask)
                nc.tensor.matmul(
                    oa[0 : DH + 1], lhsT=va, rhs=at[:, 0], start=start, stop=stop
                )
                nc.tensor.matmul(
                    ob[0 : DH + 1], lhsT=vb, rhs=at[:, 1], start=start, stop=stop
                )

            def attention_local(qT, kTloc, vloc):
                """Diagonal (own-chunk) steps; runs while the AllGather is in
                flight. Partial numerators/denominators stashed to SBUF."""
                for hp in range(NHP):
                    oa = oaccp.tile([P, CH], F32, tag="oacc")
                    ob = oaccp.tile([P, CH], F32, tag="oacc")
                    for j in range(NT):
                        attn_step(
                            hp,
                            qT,
                            kTloc[0:64, hp, ts(j, P)],
                            kTloc[64:P, hp, ts(j, P)],
                            vloc[:, j, 2 * hp],
                            vloc[:, j, 2 * hp + 1],
                            0.0,
                            trim_s[:, j],
                            oa,
                            ob,
                            start=(j == 0),
                            stop=(j == NT - 1),
                        )
                    nc.vector.tensor_copy(out=olocA[:, hp], in_=oa[0 : DH + 1])
                    nc.vector.tensor_copy(out=olocB[:, hp], in_=ob[0 : DH + 1])

            def attention_gathered(qT):
                for hp in range(NHP):
                    oa = oaccp.tile([P, CH], F32, tag="oacc")
                    ob = oaccp.tile([P, CH], F32, tag="oacc")
                    nsteps = NCG * NT
                    i = 0
                    for c in range(NCG):
                        for s in range(NT):
                            attn_step(
                                hp,
                                qT,
                                kcache[0:64, c, hp, ts(s, P)],
                                kcache[64:P, c, hp, ts(s, P)],
                                vcache[:, c, s, 2 * hp],
                                vcache[:, c, s, 2 * hp + 1],
                                mdense_s[:, c : c + 1],
                                None,
                                oa,
                                ob,
                                start=(i == 0),
                                stop=(i == nsteps - 1),
                            )
                            i += 1
                    # combine with the stashed local partials
                    nc.vector.tensor_add(
                        out=oT[0:64, hp], in0=oa[0:64], in1=olocA[0:64, hp]
                    )
                    nc.vector.tensor_add(
                        out=oTb[:, hp], in0=ob[0:64], in1=olocB[0:64, hp]
                    )
                    # denominators: psum row 64 + stashed row 64 -> fp32 ->
                    # move to partition 0 via DMA -> fast reciprocal ->
                    # broadcast across the 64 o-feature partitions
                    ls = lrecf.tile([DH + 1, 2, CH], F32, tag="lrecf")
                    nc.vector.tensor_add(
                        out=ls[64:65, 0], in0=oa[64:65], in1=olocA[64:65, hp]
                    )
                    nc.vector.tensor_add(
                        out=ls[64:65, 1], in0=ob[64:65], in1=olocB[64:65, hp]
                    )
                    la = lrecf.tile([1, 2, CH], F32, tag="lrecf2")
                    nc.sync.dma_start(la, ls[64:65])
                    lr = lrecf.tile([1, 2, CH], F32, tag="lrecf3")
                    nc.vector.reciprocal_approx_fast(lr, la)
                    lrh = lrecp.tile([1, 2, CH], BF16, tag="lrec")
                    nc.vector.tensor_copy(out=lrh, in_=lr)
                    Rb = lrecp.tile([64, 2, CH], BF16, tag="lrecb")
                    nc.gpsimd.partition_broadcast(Rb[:, 0], lrh[:, 0])
                    nc.gpsimd.partition_broadcast(Rb[:, 1], lrh[:, 1])
                    nc.vector.tensor_mul(
                        out=oT[0:64, hp], in0=oT[0:64, hp], in1=Rb[:, 0]
                    )
                    nc.vector.tensor_mul(out=oTb[:, hp], in0=oTb[:, hp], in1=Rb[:, 1])
                    nc.sync.dma_start(oT[64:P, hp], oTb[:, hp])

            def outproj(l):
                for n in range(2):
                    wot = wA.tile([P, NG, CH], BF16, tag="wA")
                    nc.sync.dma_start(
                        wot,
                        wo[l, :, ts(n, CH)].rearrange("(g p) f -> p g f", p=P),
                    )
                    for m in range(NT):
                        ps = mm_ps([P, CH], F32)
                        for g in range(NG):
                            nc.tensor.matmul(
                                ps,
                                lhsT=oT[:, g, ts(m, P)],
                                rhs=wot[:, g],
                                start=(g == 0),
                                stop=(g == NG - 1),
                            )
                        if with_bias:
                            nc.vector.tensor_tensor(
                                ps, ps, bias_bc[:, 1, ts(n, CH)], mybir.AluOpType.add
                            )
                        nc.vector.tensor_add(
                            out=xs[:, m, ts(n, CH)], in0=xs[:, m, ts(n, CH)], in1=ps
                        )

            def ffn(l, gts):
                for fs in range(NF):
                    w1t = wA.tile([P, NG, P], BF16, tag="wA")
                    nc.sync.dma_start(
                        w1t,
                        w1[l, :, ts(fs, P)].rearrange("(kd p) f -> p kd f", p=P),
                    )
                    ps = mm_ps([P, CH], F32)
                    for kd in range(NG):
                        nc.tensor.matmul(
                            ps,
                            lhsT=w1t[:, kd],
                            rhs=hT[:, kd],
                            start=(kd == 0),
                            stop=(kd == NG - 1),
                        )
                    nc.scalar.activation(
                        gts[fs // NG][:, fs % NG],
                        ps,
                        AF.Gelu,
                        bias=b1c_s[:, l, fs : fs + 1],
                    )
                for n in range(2):
                    psA = scp.tile([P, 2, CH], F32, tag="sc")
                    psB = scp.tile([P, 2, CH], F32, tag="sc")
                    pslices = [psA[:, 0], psA[:, 1], psB[:, 0], psB[:, 1]]
                    for fs in range(NF):
                        w2t = wB.tile([P, CH], BF16, tag="wB")
                        nc.sync.dma_start(w2t, w2[l, ts(fs, P), ts(n, CH)])
                        for m in range(NT):
                            nc.tensor.matmul(
                                pslices[m],
                                lhsT=gts[fs // NG][:, fs % NG, ts(m, P)],
                                rhs=w2t,
                                start=(fs == 0),
                                stop=(fs == NF - 1),
                            )
                    for m in range(NT):
                        if with_bias:
                            nc.vector.tensor_tensor(
                                pslices[m],
                                pslices[m],
                                bias_bc[:, 2, ts(n, CH)],
                                mybir.AluOpType.add,
                            )
                        nc.vector.tensor_add(
                            out=xs[:, m, ts(n, CH)],
                            in0=xs[:, m, ts(n, CH)],
                            in1=pslices[m],
                        )

            for l in range(nlayers):
                if with_bias:
                    nc.gpsimd.dma_start(
                        bias_bc,
                        brows[None, :, :].to_broadcast([P, 3, D]),
                    )
                kTloc = big.tile([P, NG, CH], BF16, tag="big", name="kTloc")
                vloc = big.tile([P, NT, H, DH + 1], BF16, tag="big", name="vloc")
                qT = big.tile([P, NHP, CH], BF16, tag="big", name="qT")
                ln_to_hT(l)
                kv_proj(l, kTloc, vloc)
                allgather_launch(l, kTloc, vloc)
                q_proj(l, qT)
                attention_local(qT, kTloc, vloc)
                gather_land(l)
                attention_gathered(qT)
                outproj(l)
                ln_to_hT(l)
                gts = [
                    big.tile([P, NG, CH], BF16, tag="big", name=f"gts{i}")
                    for i in range(4)
                ]
                ffn(l, gts)

            nc.sync.dma_start(y.rearrange("(tt p) d -> p tt d", p=P), xs)

    nc.finalize()
    return nc


# ------------------------- host side -------------------------


def _layer_norm_np(x):
    mu = x.mean(-1, keepdims=True)
    var = ((x - mu) ** 2).mean(-1, keepdims=True)
    return (x - mu) / np.sqrt(var + 1e-5)


def _sinusoidal_pe():
    pos = np.arange(T, dtype=np.float32)[:, None]
    div = np.exp(np.arange(0, D, 2, dtype=np.float32) * (-np.log(10000.0) / D))
    pe = np.zeros((T, D), dtype=np.float32)
    pe[:, 0::2] = np.sin(pos * div)
    pe[:, 1::2] = np.cos(pos * div)
    return pe


def kernel(**inputs) -> np.ndarray:
    x = np.asarray(inputs["x"], np.float32)
    lengths = np.asarray(inputs["lengths"]).astype(np.int64)
    Wqkv = np.asarray(inputs["Wqkv"], np.float32)
    bqkv = np.asarray(inputs["bqkv"], np.float32)
    Wo = np.asarray(inputs["Wo"], np.float32)
    bo = np.asarray(inputs["bo"], np.float32)
    ln0_g = np.asarray(inputs["ln0_g"], np.float32)
    ln0_b = np.asarray(inputs["ln0_b"], np.float32)
    ln1_g = np.asarray(inputs["ln1_g"], np.float32)
    ln1_b = np.asarray(inputs["ln1_b"], np.float32)
    ln2_g = np.asarray(inputs["ln2_g"], np.float32)
    ln2_b = np.asarray(inputs["ln2_b"], np.float32)
    W1 = np.asarray(inputs["W1"], np.float32)
    b1 = np.asarray(inputs["b1"], np.float32)
    W2 = np.asarray(inputs["W2"], np.float32)
    b2 = np.asarray(inputs["b2"], np.float32)

    bf16 = ml_dtypes.bfloat16

    # LN0 + positional encoding on host
    x0 = _layer_norm_np(x) * ln0_g + ln0_b + _sinusoidal_pe()[None]
    x0 = x0.astype(np.float32)

    # fold ln1/ln2 affine into the first matmul of each block
    Wqkv_eff = ln1_g[:, :, None] * Wqkv
    bqkv_eff = bqkv + np.einsum("ld,ldn->ln", ln1_b, Wqkv)
    W1_eff = ln2_g[:, :, None] * W1
    b1_eff = b1 + np.einsum("ld,ldn->ln", ln2_b, W1)

    wq_h = np.ascontiguousarray(Wqkv_eff[:, :, 0:D]).astype(bf16)
    wk_h = np.ascontiguousarray(Wqkv_eff[:, :, D : 2 * D]).astype(bf16)
    wv_h = np.ascontiguousarray(Wqkv_eff[:, :, 2 * D : 3 * D]).astype(bf16)
    wo_h = Wo.astype(bf16)
    w1_h = W1_eff.astype(bf16)
    w2_h = W2.astype(bf16)

    bq_eff = bqkv_eff[:, 0:D]
    bk_eff = bqkv_eff[:, D : 2 * D]
    bv_eff = bqkv_eff[:, 2 * D : 3 * D]

    # per-partition bias columns for Q/K: [L, P, 16] (cols 0:8 = q, 8:16 = k)
    bqk_h = np.zeros((L, P, 16), np.float32)
    for g in range(NG):
        bqk_h[:, :, g] = bq_eff[:, g * P : (g + 1) * P]
        bqk_h[:, :, 8 + g] = bk_eff[:, g * P : (g + 1) * P]
    b1c_h = np.zeros((L, P, NF), np.float32)
    for fs in range(NF):
        b1c_h[:, :, fs] = b1_eff[:, fs * P : (fs + 1) * P]

    with_bias = not (
        np.all(bv_eff == 0.0) and np.all(bo == 0.0) and np.all(b2 == 0.0)
    )
    brows_h = np.stack([bv_eff.sum(0) * 0, bo.sum(0) * 0, b2.sum(0) * 0]).astype(bf16)
    if with_bias:
        # biases are per-layer; the kernel adds the same row each layer, so the
        # general path is only valid when rows are layer-independent.
        same = (
            np.all(bv_eff == bv_eff[0]) and np.all(bo == bo[0]) and np.all(b2 == b2[0])
        )
        if not same:
            raise NotImplementedError("per-layer V/O/FFN2 biases not supported")
        brows_h = np.stack([bv_eff[0], bo[0], b2[0]]).astype(bf16)

    # static triangular masks per diagonal j-tile: [NT, P, CH]
    trim_h = np.zeros((NT, P, CH), np.float32)
    for j in range(NT):
        for i in range(NT):
            blk = trim_h[j][:, i * P : (i + 1) * P]
            if j < i:
                blk[:] = 1.0
            elif j == i:
                blk[:] = np.tril(np.ones((P, P), np.float32)).T  # keep tk <= tq
    trim_h = trim_h.astype(bf16)
    ident_h = np.eye(P, dtype=np.float32).astype(bf16)

    key = with_bias
    if key not in _CACHE:
        _CACHE[key] = _build(with_bias)
    nc = _CACHE[key]

    in_maps = []
    for core in range(8):
        b, q = core // 4, core % 4
        pos = q * CH + np.arange(CH)
        valid = (pos < lengths[b]).astype(np.float32)  # [CH]
        padcol_h = valid.reshape(NT, P).T.copy()  # [P, NT]
        posf = np.arange(NCG * CH)
        validf = (posf < lengths[b]).astype(np.float32)
        padfull_h = validf.reshape(NCG, NT, P).transpose(2, 0, 1).copy()
        mdense_h = np.zeros((P, NCG), np.float32)
        for c in range(NCG):
            if c >= q:
                mdense_h[:, c] = NEG
        m = {
            "x0": np.ascontiguousarray(x0[b, q * CH : (q + 1) * CH]),
            "wq": wq_h,
            "wk": wk_h,
            "wv": wv_h,
            "wo": wo_h,
            "w1": w1_h,
            "w2": w2_h,
            "bqk": bqk_h,
            "b1c": b1c_h,
            "mdense": mdense_h,
            "padcol": padcol_h,
            "padfull": padfull_h,
            "trim": np.ascontiguousarray(trim_h),
            "ident": ident_h,
        }
        if with_bias:
            m["brows"] = brows_h
        in_maps.append(m)

    res = run_bass_kernel_spmd(
        nc,
        in_maps,
        core_ids=list(range(8)),
        trace=bool(os.environ.get("KERNEL_TRACE")),
    )
    globals()["LAST_RESULT"] = res
    out = np.zeros((B, T, D), np.float32)
    for core in range(8):
        b, q = core // 4, core % 4
        out[b, q * CH : (q + 1) * CH] = res.results[core]["y"]
    return out
